# revision 7
# baseline (speedup 1.0000x reference)
"""Multi-head self-attention (B=2, S=2048, D=1024, H=16, causal) on 8 trn2 cores.

Sharding: core c computes heads {2c, 2c+1} for both batches (column-parallel
QKV, row-parallel O). Each core returns a partial [4096, 1024] output
(attention output of its heads projected through its slice of o_proj);
the host sums the 8 partials.

Per-core kernel (v4 — bf16, chunk-interleaved pipeline, balanced engines):
  - host supplies x pre-transposed and cast to bf16 (xT [1024, 4096]) and
    per-core weight slices pre-laid-out for SBUF, also bf16. xT is loaded
    once into 64 per-(chunk,k) tiles of [128, 512] in consumption order;
    per-chunk tiles keep batch 1's loads independent of batch 0's reads
    (whole-tile dependency tracking would otherwise serialize them).
  - per batch, the work is a pipeline over 4 query-chunks: project q/k/v
    chunk qc -> transpose V j-tiles 4qc..4qc+3 -> attention qc. The xT DMA
    stream for chunk qc+1 overlaps attention qc.
  - attention in transposed-score layout: scoresT[k, q] = K @ Q^T tiles
    (bf16, two heads packed on PE row groups), exp on ACT (scale 1/8
    fused) writing bf16 (single Exp table for the whole kernel), causal
    staircase skips invalid columns, triangular mask multiplies only
    diagonal blocks. Lag-2 software pipeline: AV for j-2 issues behind
    scores for j.
  - AV (bf16 in, fp32 accum): avT_aug[65, q] = V_aug^T @ expT; row 64 is
    the denominator.
  - normalization (part1): denominator row broadcast across 64 partitions
    with a f32r ones-outer-product matmul; r = 1/denom via DVE
    reciprocal_approx_fast (no ACT). DVE lanes are per-partition, so h1 is
    normalized at base 0 and the bf16 RESULT is DMA-shifted to partitions
    64:128 in two partition-halves (one small shift instead of two f32
    input shifts).
  - O projection (part2) bf16, K=128; PSUM->SBUF copies split DVE/ACT;
    out DMA issued per 512-col half, split again into partition-halves so
    each piece is ~2.7us of one DMA queue (descriptor-count bound).
  - scheduling: part1(b,qc) runs at j==1 of the NEXT qc's attention loop;
    its 4 O-unit part2 matmuls spread over that loop as PE fillers. The
    last qc of b1 drains in a short tail.
"""

import os
import numpy as np
from contextlib import ExitStack

import ml_dtypes

import concourse.bass as bass
import concourse.tile as tile
from concourse import bacc, mybir
from concourse.bass_utils import run_bass_kernel_spmd

F32R = mybir.dt.float32r
F32 = mybir.dt.float32
BF16 = mybir.dt.bfloat16
EXP = mybir.ActivationFunctionType.Exp
COPY = mybir.ActivationFunctionType.Copy

B, S, D = 2, 2048, 1024
NT = B * S            # 4096 tokens total
NCORES = 8
SCALE = 0.125         # 1/sqrt(64)

_BUILT = None
LAST_RESULTS = None


def _build():
    nc = bacc.Bacc("TRN2", target_bir_lowering=False, debug=False,
                   num_devices=NCORES)
    xt_d = nc.dram_tensor("xt", [D, NT], BF16, kind="ExternalInput").ap()
    wq_d = nc.dram_tensor("wq", [128, D], BF16, kind="ExternalInput").ap()
    wk_d = nc.dram_tensor("wk", [128, D], BF16, kind="ExternalInput").ap()
    wv_d = nc.dram_tensor("wv", [128, D], BF16, kind="ExternalInput").ap()
    wo_d = nc.dram_tensor("wo", [128, 1024], BF16, kind="ExternalInput").ap()
    tri_d = nc.dram_tensor("tri", [128, 128], BF16, kind="ExternalInput").ap()
    id_d = nc.dram_tensor("ident", [128, 128], BF16, kind="ExternalInput").ap()
    ones_d = nc.dram_tensor("ones", [128, 64], F32, kind="ExternalInput").ap()
    out_d = nc.dram_tensor("out", [NT, D], BF16, kind="ExternalOutput").ap()

    with tile.TileContext(nc) as tc, ExitStack() as ctx:
        consts = ctx.enter_context(tc.tile_pool(name="consts", bufs=1))
        sb = ctx.enter_context(tc.tile_pool(name="sb", bufs=1))
        ps = ctx.enter_context(tc.tile_pool(name="ps", bufs=1, space="PSUM"))

        def load_w(name, dram):
            t = consts.tile([128, D], BF16, tag=name, name=name)
            # two column-halves land on two DMA queues -> ~5.4us each
            nc.sync.dma_start(t[:, 0:512], dram[:, 0:512])
            nc.sync.dma_start(t[:, 512:1024], dram[:, 512:1024])
            return t

        # xT in 64 per-(chunk, k) tiles, issued in consumption order.
        xc = [[consts.tile([128, 512], BF16, tag=f"xc{chk}_{k}",
                           name=f"xc{chk}_{k}") for k in range(8)]
              for chk in range(8)]

        def load_x_chunk(chk):
            cs = slice(512 * chk, 512 * (chk + 1))
            for k in range(8):
                nc.sync.dma_start(xc[chk][k], xt_d[128 * k:128 * (k + 1), cs])

        wq_t = load_w("wq", wq_d)
        load_x_chunk(0)
        wk_t = load_w("wk", wk_d)
        wv_t = load_w("wv", wv_d)
        tri_t = consts.tile([128, 128], BF16, tag="tri")
        nc.sync.dma_start(tri_t, tri_d)
        id_t = consts.tile([128, 128], BF16, tag="ident")
        nc.sync.dma_start(id_t, id_d)
        # all-ones; row 64 is the lhsT of the f32r broadcast outer-product
        ones_t = consts.tile([65, 64], F32R, tag="ones")
        nc.gpsimd.dma_start(ones_t, ones_d[0:65, 0:64])
        load_x_chunk(1)
        wo_t = load_w("wo", wo_d)
        for chk in range(2, 8):
            load_x_chunk(chk)

        # ---- deferred normalize (part1) and O projection (part2) ----
        def part1(b, qc, rawf):
            """avt_all[0:64] = h0 raw av * (1/denom0); h1 normalized at
            base 0, bf16 result DMA-shifted to partitions 64:128."""
            avt_all = sb.tile([128, 512], BF16, tag="avt", bufs=4,
                              name=f"avt{b}_{qc}")
            rcp = sb.tile([64, 512], F32, tag="rcp", bufs=2,
                          name=f"rcp{b}_{qc}")
            rcp2 = sb.tile([64, 512], F32, tag="rcp2", bufs=2,
                           name=f"rcp2_{b}_{qc}")
            avlo = sb.tile([64, 512], BF16, tag="avlo", bufs=2,
                           name=f"avlo{b}_{qc}")
            cs = slice(512 * qc, 512 * (qc + 1))

            bc0 = ps.tile([64, 512], F32, tag="mm", bufs=5,
                          name=f"bc0_{b}_{qc}")
            nc.tensor.matmul(bc0, lhsT=ones_t[64:65, :],
                             rhs=rawf[0][64:65, cs], start=True, stop=True)
            bc1 = ps.tile([64, 512], F32, tag="mm", bufs=5,
                          name=f"bc1_{b}_{qc}")
            nc.tensor.matmul(bc1, lhsT=ones_t[64:65, :],
                             rhs=rawf[1][64:65, cs], start=True, stop=True)
            nc.vector.reciprocal_approx_fast(out=rcp, in_=bc0)
            nc.vector.reciprocal_approx_fast(out=rcp2, in_=bc1)
            nc.vector.tensor_mul(avt_all[0:64, :], rawf[0][0:64, cs], rcp)
            nc.vector.tensor_mul(avlo, rawf[1][0:64, cs], rcp2)
            # partition-split shift: 2 queues x 32 descriptors
            nc.sync.dma_start(avt_all[64:96, :], avlo[0:32, :])
            nc.sync.dma_start(avt_all[96:128, :], avlo[32:64, :])
            return avt_all

        def part2_unit(b, qc, avt_all, tt):
            """One token-tile of the O projection: 2 matmuls, copies split
            DVE/ACT, out DMA per chv split into partition-halves."""
            ost = sb.tile([128, 1024], BF16, tag="ost", bufs=2,
                          name=f"ost{b}_{qc}_{tt}")
            row0 = S * b + 512 * qc + 128 * tt
            for chv in range(2):
                op = ps.tile([128, 512], F32, tag="mm", bufs=5,
                             name=f"op{b}_{qc}_{tt}_{chv}")
                nc.tensor.matmul(
                    op,
                    lhsT=avt_all[:, 128 * tt:128 * (tt + 1)],
                    rhs=wo_t[:, 512 * chv:512 * (chv + 1)],
                    start=True, stop=True)
                col = slice(512 * chv, 512 * (chv + 1))
                if chv == 0:
                    nc.vector.tensor_copy(ost[:, col], op)
                else:
                    nc.scalar.activation(ost[:, col], op, COPY)
                nc.sync.dma_start(out_d[row0:row0 + 64, col],
                                  ost[0:64, col])
                nc.sync.dma_start(out_d[row0 + 64:row0 + 128, col],
                                  ost[64:128, col])

        # schedule state
        avt_ready = {}            # (b, qc) -> avt_all tile
        rawf_of = {}              # b -> rawf pair
        qt_of, kt_of, vg_of = {}, {}, {}

        def project_chunk(w_t, dst, b, chk):
            pp = ps.tile([128, 512], F32, tag="mm", bufs=5)
            for k in range(8):
                nc.tensor.matmul(
                    pp, lhsT=w_t[:, 128 * k:128 * (k + 1)],
                    rhs=xc[4 * b + chk][k],
                    start=(k == 0), stop=(k == 7))
            # ACT is idle between attention loops; copy there
            nc.scalar.activation(dst[:, 512 * chk:512 * (chk + 1)], pp, COPY)

        def attention_qc(b, qc, p1, fillers):
            """One query-chunk's attention j-loop. p1: (b', qc') part1 to
            run at j==1 (or None). fillers: (b', qc', tt) part2 O-units to
            spread across the loop at j>=2."""
            qt, kt, vg = qt_of[b], kt_of[b], vg_of[b]
            rawf = rawf_of[b]
            njt = 4 * qc + 4
            avps = [ps.tile([128, 512], F32, tag="av", bufs=2,
                            name=f"avps{b}_{qc}_{h}")
                    for h in range(2)]
            nfl = len(fillers)
            pend = []

            def do_av(j, ets):
                vs = max(0, 128 * (j - 4 * qc))
                for h in range(2):
                    nc.tensor.matmul(
                        avps[h][0:65, vs:512],
                        lhsT=vg[h][:, j, 0:65],
                        rhs=ets[h][:, vs:512],
                        start=(j == 0), stop=(j == njt - 1),
                        skip_group_check=True)

            for j in range(njt):
                vs = max(0, 128 * (j - 4 * qc))
                ets = []
                for h in range(2):
                    sc = ps.tile([128, 512], F32, tag="mm", bufs=5)
                    nc.tensor.matmul(
                        sc[:, vs:512],
                        lhsT=kt[64 * h:64 * (h + 1), 128 * j:128 * (j + 1)],
                        rhs=qt[64 * h:64 * (h + 1), 512 * qc + vs:512 * (qc + 1)],
                        start=True, stop=True)
                    et = sb.tile([128, 512], BF16, tag=f"et{h}", bufs=4)
                    nc.scalar.activation(et[:, vs:512], sc[:, vs:512],
                                         EXP, scale=SCALE)
                    if j >= 4 * qc:
                        nc.vector.tensor_mul(et[:, vs:vs + 128],
                                             et[:, vs:vs + 128], tri_t)
                    ets.append(et)
                pend.append((j, ets))
                if len(pend) > 2:   # lag-2: AV issues two iterations behind
                    do_av(*pend.pop(0))
                if j == 1 and p1 is not None:
                    avt_ready[p1] = part1(p1[0], p1[1], rawf_of[p1[0]])
                # spread O-unit fillers across j>=2 so the part1 issued at
                # j==1 of this loop is already emitted
                if j >= 2:
                    k0 = nfl * (j - 2) // (njt - 2)
                    k1 = nfl * (j - 1) // (njt - 2)
                    for k in range(k0, k1):
                        fb, fqc, ftt = fillers[k]
                        part2_unit(fb, fqc, avt_ready[(fb, fqc)], ftt)
            for args in pend:
                do_av(*args)
            # h0 copy on DVE, h1 on ACT: halves the qc-end serialization
            nc.vector.tensor_copy(rawf[0][:, 512 * qc:512 * (qc + 1)],
                                  avps[0][0:65, :])
            nc.scalar.activation(rawf[1][:, 512 * qc:512 * (qc + 1)],
                                 avps[1][0:65, :], COPY)

        for b in range(B):
            qt = sb.tile([128, S], BF16, tag="qt", bufs=2, name=f"qt{b}")
            kt = sb.tile([128, S], BF16, tag="kt", bufs=2, name=f"kt{b}")
            vt = sb.tile([128, S], BF16, tag="vt", bufs=1, name=f"vt{b}")
            vg = []
            for h in range(2):
                vgh = sb.tile([128, 16, 66], BF16, tag=f"vg{h}", bufs=2,
                              name=f"vg{b}_{h}")
                nc.gpsimd.dma_start(vgh[:, :, 64:65], ones_d[:, 0:16])
                vg.append(vgh)
            rawf = [sb.tile([65, S], F32R, tag=f"rawfull{h}", bufs=2,
                            name=f"rawf{b}_{h}")
                    for h in range(2)]
            qt_of[b], kt_of[b], vg_of[b], rawf_of[b] = qt, kt, vg, rawf

            for qc in range(4):
                # ---- projections for chunk qc ----
                project_chunk(wq_t, qt, b, qc)
                project_chunk(wk_t, kt, b, qc)
                project_chunk(wv_t, vt, b, qc)
                for j in range(4 * qc, 4 * qc + 4):
                    tp = ps.tile([128, 128], BF16, tag="mm", bufs=5)
                    nc.tensor.transpose(tp, vt[:, 128 * j:128 * (j + 1)], id_t)
                    nc.vector.tensor_copy(vg[0][:, j, 0:64], tp[:, 0:64])
                    nc.vector.tensor_copy(vg[1][:, j, 0:64], tp[:, 64:128])
                # ---- attention for chunk qc, with deferred work from the
                # ---- previous qc (cross-batch at the b boundary) ----
                if (b, qc) == (0, 0):
                    prev = None
                elif qc == 0:
                    prev = (0, 3)
                else:
                    prev = (b, qc - 1)
                fillers = ([(prev[0], prev[1], tt) for tt in range(4)]
                           if prev is not None else [])
                attention_qc(b, qc, prev, fillers)

        # tail: the last qc's normalize + O projection
        avt_ready[(1, 3)] = part1(1, 3, rawf_of[1])
        for tt in range(4):
            part2_unit(1, 3, avt_ready[(1, 3)], tt)
    nc.compile()
    return nc


def _get_built():
    global _BUILT
    if _BUILT is None:
        _BUILT = _build()
    return _BUILT


def _bf16(a):
    return np.ascontiguousarray(a.astype(ml_dtypes.bfloat16))


def _host_inputs(x, q_proj, k_proj, v_proj, o_proj):
    xth = _bf16(x.reshape(NT, D).T)
    tri = _bf16(np.triu(np.ones((128, 128), dtype=np.float32)))
    ident = _bf16(np.eye(128, dtype=np.float32))

    def wslice(w, c):
        # [p, 8k x 128m]: w_sb[p, 128k+m] = w[128c+m, 128k+p]
        a = w[128 * c:128 * (c + 1)].reshape(128, 8, 128)
        return _bf16(a.transpose(2, 1, 0).reshape(128, D))

    in_maps = []
    for c in range(NCORES):
        wo = _bf16(o_proj[:, 128 * c:128 * (c + 1)].T)
        in_maps.append(dict(
            xt=xth, wq=wslice(q_proj, c), wk=wslice(k_proj, c),
            wv=wslice(v_proj, c), wo=wo, tri=tri, ident=ident,
            ones=np.ones((128, 64), dtype=np.float32)))
    return in_maps


def kernel(**inputs):
    x = np.asarray(inputs["x"], dtype=np.float32)
    q_proj = np.asarray(inputs["q_proj"], dtype=np.float32)
    k_proj = np.asarray(inputs["k_proj"], dtype=np.float32)
    v_proj = np.asarray(inputs["v_proj"], dtype=np.float32)
    o_proj = np.asarray(inputs["o_proj"], dtype=np.float32)

    in_maps = _host_inputs(x, q_proj, k_proj, v_proj, o_proj)
    nc = _get_built()
    global LAST_RESULTS
    LAST_RESULTS = run_bass_kernel_spmd(
        nc, in_maps, core_ids=list(range(NCORES)),
        trace=bool(os.environ.get("KERNEL_TRACE")))
    acc = np.asarray(LAST_RESULTS.results[0]["out"]).astype(np.float32)
    for c in range(1, NCORES):
        acc += np.asarray(LAST_RESULTS.results[c]["out"]).astype(np.float32)
    return acc.reshape(B, S, D)


# revision 11
# speedup vs baseline: 1.0796x; 1.0796x over previous
"""Multi-head self-attention (B=2, S=2048, D=1024, H=16, causal) on 8 trn2 cores.

Sharding: core c computes heads {2c, 2c+1} for both batches (column-parallel
QKV, row-parallel O). Each core returns a partial [4096, 1024] output
(attention output of its heads projected through its slice of o_proj);
the host sums the 8 partials.

Per-core kernel (v4 — bf16, chunk-interleaved pipeline, balanced engines):
  - host supplies x pre-transposed and cast to bf16 (xT [1024, 4096]) and
    per-core weight slices pre-laid-out for SBUF, also bf16. xT is loaded
    once into 64 per-(chunk,k) tiles of [128, 512] in consumption order;
    per-chunk tiles keep batch 1's loads independent of batch 0's reads
    (whole-tile dependency tracking would otherwise serialize them).
  - per batch, the work is a pipeline over 4 query-chunks: project q/k/v
    chunk qc -> transpose V j-tiles 4qc..4qc+3 -> attention qc. The xT DMA
    stream for chunk qc+1 overlaps attention qc.
  - attention in transposed-score layout: scoresT[k, q] = K @ Q^T tiles
    (bf16, two heads packed on PE row groups), exp on ACT (scale 1/8
    fused) writing bf16 (single Exp table for the whole kernel), causal
    staircase skips invalid columns, triangular mask multiplies only
    diagonal blocks. Lag-2 software pipeline: AV for j-2 issues behind
    scores for j.
  - AV (bf16 in, fp32 accum): avT_aug[65, q] = V_aug^T @ expT; row 64 is
    the denominator.
  - normalization (part1): denominator row broadcast across 64 partitions
    with a f32r ones-outer-product matmul; r = 1/denom via DVE
    reciprocal_approx_fast (no ACT). DVE lanes are per-partition, so h1 is
    normalized at base 0 and the bf16 RESULT is DMA-shifted to partitions
    64:128 in two partition-halves (one small shift instead of two f32
    input shifts).
  - O projection (part2) bf16, K=128; PSUM->SBUF copies split DVE/ACT;
    out DMA issued per 512-col half, split again into partition-halves so
    each piece is ~2.7us of one DMA queue (descriptor-count bound).
  - scheduling: part1(b,qc) runs at j==1 of the NEXT qc's attention loop;
    its 4 O-unit part2 matmuls spread over that loop as PE fillers. The
    last qc of b1 drains in a short tail.
"""

import os
import numpy as np
from contextlib import ExitStack

import ml_dtypes

import concourse.bass as bass
import concourse.tile as tile
from concourse import bacc, mybir
from concourse.bass_utils import run_bass_kernel_spmd

F32R = mybir.dt.float32r
F32 = mybir.dt.float32
BF16 = mybir.dt.bfloat16
EXP = mybir.ActivationFunctionType.Exp
COPY = mybir.ActivationFunctionType.Copy

B, S, D = 2, 2048, 1024
NT = B * S            # 4096 tokens total
NCORES = 8
SCALE = 0.125         # 1/sqrt(64)

_BUILT = None
LAST_RESULTS = None


def _build():
    nc = bacc.Bacc("TRN2", target_bir_lowering=False, debug=False,
                   num_devices=NCORES)
    xt_d = nc.dram_tensor("xt", [D, NT], BF16, kind="ExternalInput").ap()
    wq_d = nc.dram_tensor("wq", [128, D], BF16, kind="ExternalInput").ap()
    wk_d = nc.dram_tensor("wk", [128, D], BF16, kind="ExternalInput").ap()
    wv_d = nc.dram_tensor("wv", [128, D], BF16, kind="ExternalInput").ap()
    wo_d = nc.dram_tensor("wo", [128, 1024], BF16, kind="ExternalInput").ap()
    tri_d = nc.dram_tensor("tri", [128, 128], BF16, kind="ExternalInput").ap()
    id_d = nc.dram_tensor("ident", [128, 128], BF16, kind="ExternalInput").ap()
    ones_d = nc.dram_tensor("ones", [128, 64], F32, kind="ExternalInput").ap()
    out_d = nc.dram_tensor("out", [NT, D], BF16, kind="ExternalOutput").ap()

    with tile.TileContext(nc) as tc, ExitStack() as ctx:
        consts = ctx.enter_context(tc.tile_pool(name="consts", bufs=1))
        sb = ctx.enter_context(tc.tile_pool(name="sb", bufs=1))
        ps = ctx.enter_context(tc.tile_pool(name="ps", bufs=1, space="PSUM"))

        def load_w(name, dram):
            t = consts.tile([128, D], BF16, tag=name, name=name)
            # two column-halves land on two DMA queues -> ~5.4us each
            nc.sync.dma_start(t[:, 0:512], dram[:, 0:512])
            nc.sync.dma_start(t[:, 512:1024], dram[:, 512:1024])
            return t

        # xT in 16 per-(k, batch-half) tiles. Each dma_start trigger costs
        # ~0.6us on the issuing engine (descriptors spray over all 16 DMA
        # queues regardless), so few big triggers beat many small ones.
        # b0's halves load first.
        xh = [[consts.tile([128, S], BF16, tag=f"xh{k}_{half}",
                           name=f"xh{k}_{half}") for half in range(2)]
              for k in range(8)]

        def load_x_half(half):
            cs = slice(S * half, S * (half + 1))
            for k in range(8):
                nc.sync.dma_start(xh[k][half], xt_d[128 * k:128 * (k + 1), cs])

        wq_t = load_w("wq", wq_d)
        wk_t = load_w("wk", wk_d)
        load_x_half(0)
        wv_t = load_w("wv", wv_d)
        tri_t = consts.tile([128, 128], BF16, tag="tri")
        nc.sync.dma_start(tri_t, tri_d)
        id_t = consts.tile([128, 128], BF16, tag="ident")
        nc.sync.dma_start(id_t, id_d)
        # all-ones; row 64 is the lhsT of the f32r broadcast outer-product
        ones_t = consts.tile([65, 64], F32R, tag="ones")
        nc.gpsimd.dma_start(ones_t, ones_d[0:65, 0:64])
        wo_t = load_w("wo", wo_d)
        load_x_half(1)

        # ---- deferred normalize (part1) and O projection (part2) ----
        def part1(b, qc, rawf):
            """avt_all[0:64] = h0 raw av * (1/denom0); h1 normalized at
            base 0, bf16 result DMA-shifted to partitions 64:128."""
            avt_all = sb.tile([128, 512], BF16, tag="avt", bufs=4,
                              name=f"avt{b}_{qc}")
            rcp = sb.tile([64, 512], F32, tag="rcp", bufs=2,
                          name=f"rcp{b}_{qc}")
            rcp2 = sb.tile([64, 512], F32, tag="rcp2", bufs=2,
                           name=f"rcp2_{b}_{qc}")
            avlo = sb.tile([64, 512], BF16, tag="avlo", bufs=2,
                           name=f"avlo{b}_{qc}")
            cs = slice(512 * qc, 512 * (qc + 1))

            bc0 = ps.tile([64, 512], F32, tag="mm", bufs=5,
                          name=f"bc0_{b}_{qc}")
            nc.tensor.matmul(bc0, lhsT=ones_t[64:65, :],
                             rhs=rawf[0][64:65, cs], start=True, stop=True)
            bc1 = ps.tile([64, 512], F32, tag="mm", bufs=5,
                          name=f"bc1_{b}_{qc}")
            nc.tensor.matmul(bc1, lhsT=ones_t[64:65, :],
                             rhs=rawf[1][64:65, cs], start=True, stop=True)
            nc.vector.reciprocal_approx_fast(out=rcp, in_=bc0)
            nc.vector.reciprocal_approx_fast(out=rcp2, in_=bc1)
            nc.vector.tensor_mul(avt_all[0:64, :], rawf[0][0:64, cs], rcp)
            nc.vector.tensor_mul(avlo, rawf[1][0:64, cs], rcp2)
            nc.sync.dma_start(avt_all[64:128, :], avlo)
            return avt_all

        def part2_unit(b, qc, avt_all, tt):
            """One token-tile of the O projection: 2 matmuls, copies split
            DVE/ACT, single out-DMA trigger issued from idle GpSimd."""
            ost = sb.tile([128, 1024], BF16, tag="ost", bufs=2,
                          name=f"ost{b}_{qc}_{tt}")
            row0 = S * b + 512 * qc + 128 * tt
            for chv in range(2):
                op = ps.tile([128, 512], F32, tag="mm", bufs=5,
                             name=f"op{b}_{qc}_{tt}_{chv}")
                nc.tensor.matmul(
                    op,
                    lhsT=avt_all[:, 128 * tt:128 * (tt + 1)],
                    rhs=wo_t[:, 512 * chv:512 * (chv + 1)],
                    start=True, stop=True)
                col = slice(512 * chv, 512 * (chv + 1))
                if chv == 0:
                    nc.vector.tensor_copy(ost[:, col], op)
                else:
                    nc.scalar.activation(ost[:, col], op, COPY)
            nc.gpsimd.dma_start(out_d[row0:row0 + 128, :], ost)

        # schedule state
        avt_ready = {}            # (b, qc) -> avt_all tile
        rawf_of = {}              # b -> rawf pair
        qt_of, kt_of, vg_of = {}, {}, {}

        def project_chunk(w_t, dst, b, chk):
            pp = ps.tile([128, 512], F32, tag="mm", bufs=5)
            for k in range(8):
                nc.tensor.matmul(
                    pp, lhsT=w_t[:, 128 * k:128 * (k + 1)],
                    rhs=xh[k][b][:, 512 * chk:512 * (chk + 1)],
                    start=(k == 0), stop=(k == 7))
            nc.vector.tensor_copy(dst[:, 512 * chk:512 * (chk + 1)], pp)

        def attention_qc(b, qc, p1, fillers):
            """One query-chunk's attention j-loop. p1: (b', qc') part1 to
            run at j==1 (or None). fillers: (b', qc', tt) part2 O-units to
            spread across the loop at j>=2."""
            qt, kt, vg = qt_of[b], kt_of[b], vg_of[b]
            rawf = rawf_of[b]
            njt = 4 * qc + 4
            avps = [ps.tile([128, 512], F32, tag="av", bufs=2,
                            name=f"avps{b}_{qc}_{h}")
                    for h in range(2)]
            nfl = len(fillers)
            pend = []

            def do_av(j, ets):
                vs = max(0, 128 * (j - 4 * qc))
                for h in range(2):
                    nc.tensor.matmul(
                        avps[h][0:65, vs:512],
                        lhsT=vg[h][:, j, 0:65],
                        rhs=ets[h][:, vs:512],
                        start=(j == 0), stop=(j == njt - 1),
                        skip_group_check=True)

            for j in range(njt):
                vs = max(0, 128 * (j - 4 * qc))
                ets = []
                for h in range(2):
                    sc = ps.tile([128, 512], F32, tag="mm", bufs=5)
                    nc.tensor.matmul(
                        sc[:, vs:512],
                        lhsT=kt[64 * h:64 * (h + 1), 128 * j:128 * (j + 1)],
                        rhs=qt[64 * h:64 * (h + 1), 512 * qc + vs:512 * (qc + 1)],
                        start=True, stop=True)
                    et = sb.tile([128, 512], BF16, tag=f"et{h}", bufs=4)
                    nc.scalar.activation(et[:, vs:512], sc[:, vs:512],
                                         EXP, scale=SCALE)
                    if j >= 4 * qc:
                        nc.vector.tensor_mul(et[:, vs:vs + 128],
                                             et[:, vs:vs + 128], tri_t)
                    ets.append(et)
                pend.append((j, ets))
                if len(pend) > 2:   # lag-2: AV issues two iterations behind
                    do_av(*pend.pop(0))
                if j == 1 and p1 is not None:
                    avt_ready[p1] = part1(p1[0], p1[1], rawf_of[p1[0]])
                # spread O-unit fillers across j>=2 so the part1 issued at
                # j==1 of this loop is already emitted
                if j >= 2:
                    k0 = nfl * (j - 2) // (njt - 2)
                    k1 = nfl * (j - 1) // (njt - 2)
                    for k in range(k0, k1):
                        fb, fqc, ftt = fillers[k]
                        part2_unit(fb, fqc, avt_ready[(fb, fqc)], ftt)
            for args in pend:
                do_av(*args)
            # h0 copy on DVE, h1 on ACT: halves the qc-end serialization
            nc.vector.tensor_copy(rawf[0][:, 512 * qc:512 * (qc + 1)],
                                  avps[0][0:65, :])
            nc.scalar.activation(rawf[1][:, 512 * qc:512 * (qc + 1)],
                                 avps[1][0:65, :], COPY)

        for b in range(B):
            qt = sb.tile([128, S], BF16, tag="qt", bufs=2, name=f"qt{b}")
            kt = sb.tile([128, S], BF16, tag="kt", bufs=2, name=f"kt{b}")
            vt = sb.tile([128, S], BF16, tag="vt", bufs=1, name=f"vt{b}")
            vg = []
            for h in range(2):
                vgh = sb.tile([128, 16, 66], BF16, tag=f"vg{h}", bufs=2,
                              name=f"vg{b}_{h}")
                nc.gpsimd.dma_start(vgh[:, :, 64:65], ones_d[:, 0:16])
                vg.append(vgh)
            rawf = [sb.tile([65, S], F32R, tag=f"rawfull{h}", bufs=2,
                            name=f"rawf{b}_{h}")
                    for h in range(2)]
            qt_of[b], kt_of[b], vg_of[b], rawf_of[b] = qt, kt, vg, rawf

            for qc in range(4):
                # ---- projections for chunk qc ----
                project_chunk(wq_t, qt, b, qc)
                project_chunk(wk_t, kt, b, qc)
                project_chunk(wv_t, vt, b, qc)
                for j in range(4 * qc, 4 * qc + 4):
                    tp = ps.tile([128, 128], BF16, tag="mm", bufs=5)
                    nc.tensor.transpose(tp, vt[:, 128 * j:128 * (j + 1)], id_t)
                    nc.vector.tensor_copy(vg[0][:, j, 0:64], tp[:, 0:64])
                    nc.vector.tensor_copy(vg[1][:, j, 0:64], tp[:, 64:128])
                # ---- attention for chunk qc, with deferred work from the
                # ---- previous qc (cross-batch at the b boundary) ----
                if (b, qc) == (0, 0):
                    prev = None
                elif qc == 0:
                    prev = (0, 3)
                else:
                    prev = (b, qc - 1)
                fillers = ([(prev[0], prev[1], tt) for tt in range(4)]
                           if prev is not None else [])
                attention_qc(b, qc, prev, fillers)

        # tail: the last qc's normalize + O projection
        avt_ready[(1, 3)] = part1(1, 3, rawf_of[1])
        for tt in range(4):
            part2_unit(1, 3, avt_ready[(1, 3)], tt)
    nc.compile()
    return nc


def _get_built():
    global _BUILT
    if _BUILT is None:
        _BUILT = _build()
    return _BUILT


def _bf16(a):
    return np.ascontiguousarray(a.astype(ml_dtypes.bfloat16))


def _host_inputs(x, q_proj, k_proj, v_proj, o_proj):
    xth = _bf16(x.reshape(NT, D).T)
    tri = _bf16(np.triu(np.ones((128, 128), dtype=np.float32)))
    ident = _bf16(np.eye(128, dtype=np.float32))

    def wslice(w, c):
        # [p, 8k x 128m]: w_sb[p, 128k+m] = w[128c+m, 128k+p]
        a = w[128 * c:128 * (c + 1)].reshape(128, 8, 128)
        return _bf16(a.transpose(2, 1, 0).reshape(128, D))

    in_maps = []
    for c in range(NCORES):
        wo = _bf16(o_proj[:, 128 * c:128 * (c + 1)].T)
        in_maps.append(dict(
            xt=xth, wq=wslice(q_proj, c), wk=wslice(k_proj, c),
            wv=wslice(v_proj, c), wo=wo, tri=tri, ident=ident,
            ones=np.ones((128, 64), dtype=np.float32)))
    return in_maps


def kernel(**inputs):
    x = np.asarray(inputs["x"], dtype=np.float32)
    q_proj = np.asarray(inputs["q_proj"], dtype=np.float32)
    k_proj = np.asarray(inputs["k_proj"], dtype=np.float32)
    v_proj = np.asarray(inputs["v_proj"], dtype=np.float32)
    o_proj = np.asarray(inputs["o_proj"], dtype=np.float32)

    in_maps = _host_inputs(x, q_proj, k_proj, v_proj, o_proj)
    nc = _get_built()
    global LAST_RESULTS
    LAST_RESULTS = run_bass_kernel_spmd(
        nc, in_maps, core_ids=list(range(NCORES)),
        trace=bool(os.environ.get("KERNEL_TRACE")))
    acc = np.asarray(LAST_RESULTS.results[0]["out"]).astype(np.float32)
    for c in range(1, NCORES):
        acc += np.asarray(LAST_RESULTS.results[c]["out"]).astype(np.float32)
    return acc.reshape(B, S, D)
